# revision 1
# baseline (speedup 1.0000x reference)
"""Self-contained 8-core Trainium2 Bass kernel for the 3-layer RGCN
entity-classification problem (N=100000 nodes, E=1000000 edges, H=64,
R=90 relations, B=8 bases, OUT=16).

Per layer:
  W_r  = sum_b coeff[r,b] * bases[b]               (computed on device)
  m_e  = norm_e * (x[src_e] @ W_{etype_e})         (pass A, etype-grouped tiles)
  h[v] = relu(sum_{e: dst_e=v} m_e + bias)         (pass B, dst-window one-hot matmuls)

Edges are sharded by dst-range (core c owns nodes [c*12500,(c+1)*12500)).
Pass A is ordered (src-chunk, etype): each 128-edge tile uses one relation
weight (segmented matmuls at 64-granularity for group boundaries) and each
dma_gather call reads from one <=32768-row x-chunk (int16 index limit).
Messages go to HBM partition-major; pass B gathers them back in (rank-chunk,
dst-window) order and segment-sums via one-hot matmuls accumulating in PSUM,
spilling per-group into an SBUF h accumulator.  Node features (and the h
produced by each layer) are AllGathered across the 8 cores.  Tile counts per
group are padded to the max over cores so a single SPMD program serves all
cores; per-core index/metadata arrays arrive as kernel inputs.
"""

import os
import sys

for _p in ("/opt/trn_rl_repo",
           os.path.expanduser("~/.axon_site/_ro/trn_rl_repo")):
    if os.path.isdir(_p) and _p not in sys.path:
        sys.path.insert(0, _p)

import numpy as np


class _Schedule:
    pass


def _prepare(src, dst, etype, norm, *, N, R, NC=8, WN=125, XCHN=4, MCH=32768,
             GCALL=8):
    src = np.asarray(src).astype(np.int64).ravel()
    dst = np.asarray(dst).astype(np.int64).ravel()
    etype = np.asarray(etype).astype(np.int64).ravel()
    norm = np.asarray(norm).astype(np.float32).ravel()
    NPC = N // NC
    assert NPC * NC == N
    NW = NPC // WN
    assert NW * WN == NPC
    XCH = (N + XCHN - 1) // XCHN
    assert XCH <= 32768

    core = dst // NPC
    gA_all = (src // XCH) * R + etype
    NGA = XCHN * R

    # ---- pass A grouping: (src-chunk, etype), sizes padded to 64
    ordA, cntA = [], np.zeros((NC, NGA), np.int64)
    for c in range(NC):
        e = np.flatnonzero(core == c)
        eo = e[np.argsort(gA_all[e], kind="stable")]
        ordA.append(eo)
        g, n = np.unique(gA_all[eo], return_counts=True)
        cntA[c, g] = n
    SZA = np.zeros(NGA, np.int64)
    mx = cntA.max(axis=0)
    SZA[mx > 0] = (mx[mx > 0] + 63) // 64 * 64
    for ch in range(XCHN):
        sl = slice(ch * R, (ch + 1) * R)
        extra = (-SZA[sl].sum()) % 128
        if extra:
            nz = np.flatnonzero(SZA[sl])
            SZA[ch * R + (nz[-1] if len(nz) else 0)] += extra
    NTA = int(SZA.sum()) // 128
    EA = NTA * 128

    gstartA = np.concatenate([[0], np.cumsum(SZA)])
    segA = [[] for _ in range(NTA)]

    def _pieces(a, e):
        out = []
        while a < e:
            step = e - a if a == 0 and e == 128 else 64
            out.append((a, a + step))
            a += step
        return out

    for g in range(NGA):
        a, bnd = gstartA[g], gstartA[g + 1]
        while a < bnd:
            t = a // 128
            e = min(bnd, (t + 1) * 128)
            for (pa, pe) in _pieces(int(a - t * 128), int(e - t * 128)):
                segA[t].append((pa, pe, int(g % R)))
            a = e

    callsA = []
    t = 0
    for ch in range(XCHN):
        nt = int(SZA[ch * R:(ch + 1) * R].sum()) // 128
        o = 0
        while o < nt:
            n = min(GCALL, nt - o)
            callsA.append((ch, t + o, n))
            o += n
        t += nt

    gidxA = np.zeros((NC, EA), np.int16)
    normA = np.zeros((NC, EA), np.float32)
    slotA = []
    for c in range(NC):
        eo = ordA[c]
        g = gA_all[eo]
        pos = np.zeros(len(eo), np.int64)
        gg, idx0, n = np.unique(g, return_index=True, return_counts=True)
        for gi, i0, nn in zip(gg, idx0, n):
            pos[i0:i0 + nn] = gstartA[gi] + np.arange(nn)
        gidxA[c, pos] = (src[eo] - (g // R) * XCH).astype(np.int16)
        normA[c, pos] = norm[eo]
        rank = (pos % 128) * NTA + (pos // 128)   # m-buffer row (part-major)
        slotA.append((eo, rank))

    # ---- pass B grouping: (rank-chunk q, window w), q-major, sizes pad 64
    Q = (EA + MCH - 1) // MCH
    NGB = Q * NW
    cntB = np.zeros((NC, NGB), np.int64)
    ordB = []
    for c in range(NC):
        eo, rank = slotA[c]
        q = rank // MCH
        w = (dst[eo] - c * NPC) // WN
        wloc = (dst[eo] - c * NPC) % WN
        gB = q * NW + w
        o = np.argsort(gB, kind="stable")
        ordB.append((rank[o], wloc[o], gB[o]))
        g, n = np.unique(gB[o], return_counts=True)
        cntB[c, g] = n
    SZB = np.zeros(NGB, np.int64)
    mx = cntB.max(axis=0)
    SZB[mx > 0] = (mx[mx > 0] + 63) // 64 * 64
    for q in range(Q):
        sl = slice(q * NW, (q + 1) * NW)
        extra = (-SZB[sl].sum()) % 128
        if extra:
            nz = np.flatnonzero(SZB[sl])
            SZB[q * NW + (nz[-1] if len(nz) else 0)] += extra
    NTB = int(SZB.sum()) // 128
    EB = NTB * 128

    gstartB = np.concatenate([[0], np.cumsum(SZB)])
    segB = [[] for _ in range(NTB)]
    for g in range(NGB):
        a, bnd = gstartB[g], gstartB[g + 1]
        first = True
        while a < bnd:
            t = a // 128
            e = min(bnd, (t + 1) * 128)
            segB[t].append([int(a - t * 128), int(e - t * 128), int(g % NW),
                            int(g), first, e == bnd])
            first = False
            a = e

    callsB = []
    t = 0
    for q in range(Q):
        nt = int(SZB[q * NW:(q + 1) * NW].sum()) // 128
        o = 0
        while o < nt:
            n = min(GCALL, nt - o)
            callsB.append((q, t + o, n))
            o += n
        t += nt

    gidxB = np.zeros((NC, EB), np.int16)
    dstlocB = np.full((NC, EB), -1, np.float32)
    for c in range(NC):
        pos, wloc, gB = ordB[c]
        slot = np.zeros(len(pos), np.int64)
        gg, idx0, n = np.unique(gB, return_index=True, return_counts=True)
        for gi, i0, nn in zip(gg, idx0, n):
            slot[i0:i0 + nn] = gstartB[gi] + np.arange(nn)
        gidxB[c, slot] = (pos % MCH).astype(np.int16)
        dstlocB[c, slot] = wloc

    s = _Schedule()
    s.N, s.R, s.NC, s.NPC, s.WN, s.NW = N, R, NC, NPC, WN, NW
    s.XCH, s.XCHN, s.MCH, s.Q = XCH, XCHN, MCH, Q
    s.EA, s.EB, s.NTA, s.NTB, s.GC = EA, EB, NTA, NTB, GCALL
    s.segA, s.segB = segA, segB
    s.callsA, s.callsB = callsA, callsB
    s.gidxA = _pm_idx(gidxA, callsA, GCALL)
    s.normA = _pm_stream(normA, callsA, GCALL)
    s.gidxB = _pm_idx(gidxB, callsB, GCALL)
    s.dstlocB = _pm_stream(dstlocB, callsB, GCALL)
    return s


def _pm_idx(a, calls, GC):
    """[NC, E] -> [NC, 16, ncalls*GC*8] wrapped idx planes (idx i of call at
    [i%16, call*GC*8 + i//16]); replicated to 128 partitions on device."""
    NC = a.shape[0]
    out = np.zeros((NC, 16, len(calls) * GC * 8), a.dtype)
    for ci, (_, t0, nt) in enumerate(calls):
        blk = a[:, t0 * 128:(t0 + nt) * 128]
        w = blk.reshape(NC, nt * 8, 16).transpose(0, 2, 1)
        out[:, :, ci * GC * 8:ci * GC * 8 + nt * 8] = w
    return out


def _pm_stream(a, calls, GC):
    """[NC, E] -> [NC, 128, ncalls*GC]: edge (t*128+p) of call at [p, call*GC+t]."""
    NC = a.shape[0]
    out = np.zeros((NC, 128, len(calls) * GC), a.dtype)
    for ci, (_, t0, nt) in enumerate(calls):
        blk = a[:, t0 * 128:(t0 + nt) * 128]
        out[:, :, ci * GC:ci * GC + nt] = blk.reshape(NC, nt, 128).transpose(0, 2, 1)
    return out


def _build(s, H=64, OUT=16, B=8, biases=None, collectives=True):
    import concourse.bass as bass  # noqa: F401
    import concourse.bacc as bacc
    import concourse.mybir as mybir
    from concourse.tile import TileContext

    f32 = mybir.dt.float32
    i16 = mybir.dt.int16
    AF = mybir.ActivationFunctionType
    NC, EA, EB, GC = s.NC, s.EA, s.EB, s.GC
    R, WN, NW = s.R, s.WN, s.NW
    nA, nB = len(s.callsA), len(s.callsB)
    Odims = [H, H, H]           # layer 2 computed at width H (zero-padded)
    use_bias = [biases is not None and np.asarray(biases[l]).any()
                for l in range(3)]

    nc = bacc.Bacc(None, target_bir_lowering=False)

    fshard = nc.declare_dram_parameter("fshard", [s.NPC, H], f32, isOutput=False)
    feats = nc.dram_tensor("feats", [s.N, H], f32, addr_space="Shared")
    fbounce = nc.dram_tensor("fbounce", [s.NPC, H], f32)
    coeffT = [nc.declare_dram_parameter(f"coeffT{l}", [B, R], f32, isOutput=False) for l in range(3)]
    bases2d = [nc.declare_dram_parameter(f"bases2d{l}", [B, H * Odims[l]], f32, isOutput=False) for l in range(3)]
    biasr = [nc.declare_dram_parameter(f"biasr{l}", [1, H], f32, isOutput=False) for l in range(3)]
    gidxA = nc.declare_dram_parameter("gidxA", [16, nA * GC * 8], i16, isOutput=False)
    normA = nc.declare_dram_parameter("normA", [128, nA * GC], f32, isOutput=False)
    gidxB = nc.declare_dram_parameter("gidxB", [16, nB * GC * 8], i16, isOutput=False)
    dstlocB = nc.declare_dram_parameter("dstlocB", [128, nB * GC], f32, isOutput=False)
    outs = nc.declare_dram_parameter("outs", [s.NPC, OUT], f32, isOutput=True)

    mtil = nc.dram_tensor("mtil", [128, EA // 128, H], f32)
    hslice = [nc.dram_tensor(f"hslice{l}", [s.NPC, H], f32) for l in range(2)]
    hfull = [nc.dram_tensor(f"hfull{l}", [s.N, H], f32, addr_space="Shared") for l in range(2)]

    # ---- replicate/broadcast node features to every core
    with nc.semaphore("fgdma") as fg_dma, nc.semaphore("fgcc") as fg_cc:
        if collectives:
            nc.gpsimd.dma_start(out=fbounce[:], in_=fshard[:]).then_inc(fg_dma, 16)
            nc.gpsimd.wait_ge(fg_dma, 16)
            nc.gpsimd.collective_compute(
                "AllGather", mybir.AluOpType.bypass,
                replica_groups=[list(range(NC))],
                ins=[fbounce[:]], outs=[feats[:]],
            ).then_inc(fg_cc, 1)
            nc.gpsimd.wait_ge(fg_cc, 1)
        else:
            nc.gpsimd.dma_start(out=feats[0:s.NPC, :], in_=fshard[:]).then_inc(fg_dma, 16)
            nc.gpsimd.wait_ge(fg_dma, 16)
    nc.all_engine_barrier()

    for l in range(3):
        O = Odims[l]
        OW = OUT if l == 2 else H
        xsrc = feats if l == 0 else hfull[l - 1]
        with TileContext(nc) as tc:
            with (
                tc.tile_pool(name="const", bufs=1) as pconst,
                tc.tile_pool(name="wstage", bufs=1) as pw,
                tc.tile_pool(name="wpsum", bufs=1, space="PSUM") as pwps,
                tc.tile_pool(name="idx", bufs=2) as pidx,
                tc.tile_pool(name="gx", bufs=3) as pgx,
                tc.tile_pool(name="meta", bufs=2) as pmeta,
                tc.tile_pool(name="xt", bufs=3) as pxt,
                tc.tile_pool(name="xtp", bufs=2, space="PSUM") as pxtps,
                tc.tile_pool(name="mps", bufs=2, space="PSUM") as pmps,
                tc.tile_pool(name="mout", bufs=3) as pmout,
                tc.tile_pool(name="oh", bufs=4) as poh,
                tc.tile_pool(name="hps", bufs=2, space="PSUM") as phps,
                tc.tile_pool(name="hacc", bufs=1) as phacc,
                tc.tile_pool(name="ost", bufs=2) as post,
            ):
                # constants: iota row [0..WN), identity matrix for PE transpose
                iota = pconst.tile([128, WN], f32, tag="iota")
                nc.gpsimd.iota(iota[:], [[1, WN]], base=0, channel_multiplier=0,
                               allow_small_or_imprecise_dtypes=True)
                iop = pconst.tile([128, 1], f32, tag="iop")
                nc.gpsimd.iota(iop[:], [[0, 1]], base=0, channel_multiplier=1,
                               allow_small_or_imprecise_dtypes=True)
                iof = pconst.tile([128, 128], f32, tag="iof")
                nc.gpsimd.iota(iof[:], [[1, 128]], base=0, channel_multiplier=0,
                               allow_small_or_imprecise_dtypes=True)
                ident = pconst.tile([128, 128], f32, tag="ident")
                nc.vector.tensor_scalar(ident[:], iof[:], iop[:], None,
                                        op0=mybir.AluOpType.is_equal)
                if use_bias[l]:
                    brow = pconst.tile([128, H], f32, tag="brow")
                    nc.sync.dma_start(out=brow[:1, :], in_=biasr[l][:])
                    bbc = pconst.tile([128, H], f32, tag="bbc")
                    nc.gpsimd.partition_broadcast(bbc[:], brow[:1, :])

                # W_sb[i, r, o] = sum_b bases[b,i,o]*coeff[r,b]; both halves
                ct = pw.tile([128, R], f32, tag="ct")
                nc.sync.dma_start(out=ct[:B, :], in_=coeffT[l][:])
                b2 = pw.tile([128, H * O], f32, tag="b2")
                nc.sync.dma_start(out=b2[:B, :], in_=bases2d[l][:])
                b2v = b2.rearrange("p (i o) -> p i o", o=O)
                W_sb = pw.tile([2 * H, R, O], f32, tag="wsb")
                for o in range(O):
                    wp = pwps.tile([128, R], f32, tag="wp")
                    for half in range(2):
                        nc.tensor.matmul(wp[half * H:half * H + H, :R],
                                         b2v[:B, :, o], ct[:B, :],
                                         start=True, stop=True)
                    nc.scalar.activation(W_sb[:, :, o], wp[:, :R], AF.Copy)

                hacc = phacc.tile([128, NW * H], f32, tag="hacc")
                nc.gpsimd.memset(hacc[:], 0.0)

                # ---- pass A
                KB = 16
                itb = nrmb = None
                for ci, (ch, t0, nt) in enumerate(s.callsA):
                    if ci % KB == 0:
                        n_in_b = min(KB, nA - ci)
                        itb = pidx.tile([128, KB * GC * 8], i16, tag="it")
                        for rp in range(8):
                            nc.sync.dma_start(
                                out=itb[rp * 16:(rp + 1) * 16, :n_in_b * GC * 8],
                                in_=gidxA[:, ci * GC * 8:(ci + n_in_b) * GC * 8])
                        nrmb = pmeta.tile([128, KB * GC], f32, tag="nrm")
                        nc.sync.dma_start(
                            out=nrmb[:, :n_in_b * GC],
                            in_=normA[:, ci * GC:(ci + n_in_b) * GC])
                    it = itb[:, (ci % KB) * GC * 8:(ci % KB) * GC * 8 + nt * 8]
                    nrm = nrmb[:, (ci % KB) * GC:(ci % KB) * GC + GC]
                    gt = pgx.tile([128, GC, H], f32, tag="gt")
                    rows = min(s.XCH, s.N - ch * s.XCH)
                    nc.gpsimd.dma_gather(
                        gt[:, :nt, :], xsrc[ch * s.XCH:ch * s.XCH + rows],
                        it, num_idxs=nt * 128, num_idxs_reg=nt * 128,
                        elem_size=H)
                    ms = pmout.tile([128, GC, H], f32, tag="ms")
                    for k in range(0, nt, 2):
                        n2 = min(2, nt - k)
                        xtp = pxtps.tile([128, 128], f32, tag="xtp")
                        if n2 == 2:
                            nc.tensor.transpose(
                                xtp[:],
                                gt[:, k:k + 2, :].rearrange("p a b -> p (a b)"),
                                ident[:])
                        else:
                            nc.tensor.transpose(xtp[:H, :], gt[:, k, :], ident[:])
                        xt = pxt.tile([128, 128], f32, tag="xt")
                        nc.vector.tensor_copy(xt[:n2 * H, :], xtp[:n2 * H, :])
                        for j in range(n2):
                            t = t0 + k + j
                            mp = pmps.tile([128, H], f32, tag="mp")
                            for (a, e, rel) in s.segA[t]:
                                nc.tensor.matmul(
                                    mp[a:e, :O],
                                    xt[j * H:(j + 1) * H, a:e],
                                    W_sb[j * H:(j + 1) * H, rel, :],
                                    start=True, stop=True)
                            nc.scalar.activation(ms[:, k + j, :O], mp[:, :O],
                                                 AF.Copy,
                                                 scale=nrm[:, k + j:k + j + 1])
                    nc.sync.dma_start(out=mtil[:, t0:t0 + nt, :],
                                      in_=ms[:, :nt, :])

                # ---- pass B
                hp_of = {}
                itb = dlb = None
                for ci, (q, t0, nt) in enumerate(s.callsB):
                    if ci % KB == 0:
                        n_in_b = min(KB, nB - ci)
                        itb = pidx.tile([128, KB * GC * 8], i16, tag="it")
                        for rp in range(8):
                            nc.sync.dma_start(
                                out=itb[rp * 16:(rp + 1) * 16, :n_in_b * GC * 8],
                                in_=gidxB[:, ci * GC * 8:(ci + n_in_b) * GC * 8])
                        dlb = pmeta.tile([128, KB * GC], f32, tag="dl")
                        nc.sync.dma_start(
                            out=dlb[:, :n_in_b * GC],
                            in_=dstlocB[:, ci * GC:(ci + n_in_b) * GC])
                    it = itb[:, (ci % KB) * GC * 8:(ci % KB) * GC * 8 + nt * 8]
                    dl = dlb[:, (ci % KB) * GC:(ci % KB) * GC + GC]
                    mt = pgx.tile([128, GC, H], f32, tag="gt")
                    rows = min(s.MCH, EA - q * s.MCH)
                    mflat = mtil.ap().rearrange("p a f -> (p a) f")
                    nc.gpsimd.dma_gather(
                        mt[:, :nt, :], mflat[q * s.MCH:q * s.MCH + rows],
                        it, num_idxs=nt * 128, num_idxs_reg=nt * 128,
                        elem_size=H)
                    for k in range(nt):
                        t = t0 + k
                        oh = poh.tile([128, WN], f32, tag="oh")
                        nc.vector.tensor_scalar(oh[:], iota[:], dl[:, k:k + 1],
                                                None, op0=mybir.AluOpType.is_equal)
                        for (a, e, w, g, first, last) in s.segB[t]:
                            if first:
                                hp_of[g] = phps.tile([128, H], f32, tag="hp",
                                                     name=f"hp{l}_{g}")
                            hp = hp_of[g]
                            nc.tensor.matmul(hp[:WN, :O], oh[a:e, :],
                                             mt[a:e, k, :O],
                                             start=first, stop=last)
                            if last:
                                nc.vector.tensor_tensor(
                                    out=hacc[:WN, w * H:w * H + O],
                                    in0=hp[:WN, :O],
                                    in1=hacc[:WN, w * H:w * H + O],
                                    op=mybir.AluOpType.add)
                                del hp_of[g]

                # ---- flush: (+bias) relu / copy, write h slice or output
                WB = next(d for d in (5, 4, 2, 1) if NW % d == 0)
                for w0 in range(0, NW, WB):
                    ob = post.tile([128, WB, H], f32, tag="ob")
                    for w in range(w0, w0 + WB):
                        hsl = hacc[:WN, w * H:w * H + OW]
                        if use_bias[l]:
                            nc.vector.tensor_tensor(
                                out=hsl, in0=hsl, in1=bbc[:WN, :OW],
                                op=mybir.AluOpType.add)
                        nc.scalar.activation(
                            ob[:WN, w - w0, :OW], hsl,
                            AF.Relu if l < 2 else AF.Copy)
                    dstt = outs if l == 2 else hslice[l]
                    nc.sync.dma_start(
                        out=dstt.ap()[w0 * WN:(w0 + WB) * WN, :]
                            .rearrange("(a p) f -> p a f", p=WN),
                        in_=ob[:WN, :, :OW])

        if l < 2:
            with nc.semaphore(f"ccsem{l}") as cc_sem:
                if collectives:
                    nc.gpsimd.collective_compute(
                        "AllGather", mybir.AluOpType.bypass,
                        replica_groups=[list(range(NC))],
                        ins=[hslice[l][:]], outs=[hfull[l][:]],
                    ).then_inc(cc_sem)
                    nc.gpsimd.wait_ge(cc_sem, 1)
                else:
                    nc.gpsimd.dma_start(out=hfull[l][0:s.NPC, :],
                                        in_=hslice[l][:]).then_inc(cc_sem, 16)
                    nc.gpsimd.wait_ge(cc_sem, 16)
            nc.all_engine_barrier()

    nc.finalize()
    return nc


def _padded_bases(b, H):
    b = np.asarray(b, np.float32)
    B, Hb, O = b.shape
    if O < H:
        bp = np.zeros((B, Hb, H), np.float32)
        bp[:, :, :O] = b
        b = bp
    return np.ascontiguousarray(b.reshape(B, -1))


def _make_in_maps(inputs, s, H):
    feats = np.ascontiguousarray(np.asarray(inputs["feats"], np.float32))
    B = np.asarray(inputs["coeff0"]).shape[1]
    base = {
        **{f"coeffT{l}": np.ascontiguousarray(
            np.asarray(inputs[f"coeff{l}"], np.float32).T) for l in range(3)},
        **{f"bases2d{l}": _padded_bases(inputs[f"bases{l}"], H) for l in range(3)},
        **{f"biasr{l}": np.pad(np.asarray(inputs[f"bias{l}"], np.float32),
                               (0, H - len(np.asarray(inputs[f"bias{l}"]))),
                               ).reshape(1, H) for l in range(3)},
    }
    return [
        {**base,
         "fshard": feats[c * s.NPC:(c + 1) * s.NPC],
         "gidxA": s.gidxA[c], "normA": s.normA[c],
         "gidxB": s.gidxB[c], "dstlocB": s.dstlocB[c]}
        for c in range(s.NC)
    ]


_CACHE = {}


def _get_compiled(inputs):
    feats = np.asarray(inputs["feats"], np.float32)
    N, H = feats.shape
    OUT = np.asarray(inputs["bases2"]).shape[2]
    R, B = np.asarray(inputs["coeff0"]).shape
    key = (N, H, OUT, R, B)
    if key not in _CACHE:
        s = _prepare(inputs["src"], inputs["dst"], inputs["etype"],
                     inputs["norm"], N=N, R=R, NC=8)
        biases = [np.asarray(inputs[f"bias{l}"], np.float32) for l in range(3)]
        nc = _build(s, H=H, OUT=OUT, B=B, biases=biases, collectives=True)
        _CACHE[key] = (s, nc)
    return _CACHE[key]


def kernel(**inputs):
    """Full-input, full-output 3-layer RGCN on 8 NeuronCores."""
    from concourse.bass_utils import run_bass_kernel_spmd

    H = np.asarray(inputs["feats"]).shape[1]
    s, nc = _get_compiled(inputs)
    in_maps = _make_in_maps(inputs, s, H)
    res = run_bass_kernel_spmd(nc, in_maps, list(range(s.NC)))
    return np.concatenate([res.results[c]["outs"] for c in range(s.NC)], axis=0)


if __name__ == "__main__":
    # quick self-check with random data of the real problem shape
    rng = np.random.default_rng(0)
    N, E, H, OUT, R, B = 100000, 1000000, 64, 16, 90, 8
    inputs = dict(
        feats=rng.standard_normal((N, H)).astype(np.float32),
        src=rng.integers(0, N, E), dst=rng.integers(0, N, E),
        etype=rng.integers(0, R, E),
        norm=rng.random((E, 1)).astype(np.float32),
    )
    for l, o in enumerate([H, H, OUT]):
        inputs[f"coeff{l}"] = (rng.standard_normal((R, B)) / np.sqrt(B)).astype(np.float32)
        inputs[f"bases{l}"] = (rng.standard_normal((B, H, o)) / np.sqrt(H)).astype(np.float32)
        inputs[f"bias{l}"] = np.zeros(o, np.float32)
    out = kernel(**inputs)
    print("kernel out", out.shape, out.dtype, float(np.abs(out).max()))



# revision 7
# speedup vs baseline: 1.4002x; 1.4002x over previous
"""Self-contained 8-core Trainium2 Bass kernel for the 3-layer RGCN
entity-classification problem (N=100000 nodes, E=1000000 edges, H=64,
R=90 relations, B=8 bases, OUT=16).

Strategy (single-pass, one gather per edge per layer):
  Edges are sharded by dst range (core c owns nodes [c*12500,(c+1)*12500))
  and ordered by (dst-window, src-chunk); per-(window,chunk) group sizes are
  padded to multiples of 128 and to the max over cores, so every 128-edge
  tile belongs to exactly one dst-window and one src-chunk and a single SPMD
  program serves all 8 cores.  Per tile:
    xt   = transpose(x[src])                       (PE transpose, f32)
    P    = xt.T @ [bases_0 | ... | bases_7]        (one 512-wide bf16 matmul)
    T    = P * cn  (cn[e,b] = coeff[etype_e,b]*norm_e, streamed; DVE, bf16 out)
    hp  += onehot(dstloc).T @ T                    (512-wide bf16 matmul; PSUM
                                                    chain spans the window)
  On a window's last tile: h[w] = relu(reduce_b(hp)) via a strided DVE reduce
  and an Act copy straight into the output staging tile.  This removes the
  baseline's second per-edge gather (messages never round-trip through HBM)
  and replaces per-relation segmented matmuls with relation-independent basis
  matmuls.  h is AllGathered between layers; feats arrive replicated per core
  so there is no initial collective.
"""

import os
import sys

for _p in ("/opt/trn_rl_repo",
           os.path.expanduser("~/.axon_site/_ro/trn_rl_repo")):
    if os.path.isdir(_p) and _p not in sys.path:
        sys.path.insert(0, _p)

import numpy as np


class _Schedule:
    pass


def _prepare(src, dst, *, N, NC=8, WN=125, XCHN=4, GCALL=8):
    src = np.asarray(src).astype(np.int64).ravel()
    dst = np.asarray(dst).astype(np.int64).ravel()
    NPC = N // NC
    assert NPC * NC == N
    NW = NPC // WN
    assert NW * WN == NPC
    XCH = (N + XCHN - 1) // XCHN
    assert XCH <= 32767

    core = dst // NPC
    NG = NW * XCHN                       # (window, chunk), window-major
    g_all = ((dst % NPC) // WN) * XCHN + (src // XCH)

    ordc, cnt = [], np.zeros((NC, NG), np.int64)
    for c in range(NC):
        e = np.flatnonzero(core == c)
        eo = e[np.argsort(g_all[e], kind="stable")]
        ordc.append(eo)
        g, n = np.unique(g_all[eo], return_counts=True)
        cnt[c, g] = n
    SZ = cnt.max(axis=0).astype(np.int64)
    SZ[SZ > 0] = (SZ[SZ > 0] + 127) // 128 * 128   # whole tiles per group
    NT = int(SZ.sum()) // 128
    EA = NT * 128

    gstart = np.concatenate([[0], np.cumsum(SZ)])
    # window w covers tiles [gstart[w*XCHN]//128, gstart[(w+1)*XCHN]//128)
    wst = gstart[::XCHN] // 128
    window_of = np.repeat(np.arange(NW), np.diff(wst))
    assert len(window_of) == NT

    calls = []
    for g in range(NG):
        ch = g % XCHN
        t0, nt_g = int(gstart[g]) // 128, int(SZ[g]) // 128
        o = 0
        while o < nt_g:
            n = min(GCALL, nt_g - o)
            calls.append((ch, t0 + o, n))
            o += n

    gidx = np.zeros((NC, EA), np.int16)
    dstloc = np.full((NC, EA), -1, np.float32)
    slots = []
    for c in range(NC):
        eo = ordc[c]
        g = g_all[eo]
        pos = np.zeros(len(eo), np.int64)
        gg, idx0, n = np.unique(g, return_index=True, return_counts=True)
        for gi, i0, nn in zip(gg, idx0, n):
            pos[i0:i0 + nn] = gstart[gi] + np.arange(nn)
        gidx[c, pos] = (src[eo] % XCH).astype(np.int16)
        dstloc[c, pos] = ((dst[eo] % NPC) % WN).astype(np.float32)
        slots.append((eo, pos))

    s = _Schedule()
    s.N, s.NC, s.NPC, s.WN, s.NW = N, NC, NPC, WN, NW
    s.XCH, s.XCHN, s.GC = XCH, XCHN, GCALL
    s.EA, s.NT = EA, NT
    s.calls, s.slots = calls, slots
    s.window_of, s.wst = window_of, wst
    s.gidxS = _pm_idx(gidx, calls, GCALL)
    s.dlS = _pm_stream(dstloc[:, :, None], calls, GCALL)
    return s


def _pm_idx(a, calls, GC):
    """[NC, E] -> [NC, 16, ncalls*GC*8] wrapped idx planes (idx i of call at
    [i%16, call*GC*8 + i//16]); replicated to 128 partitions on device."""
    NC = a.shape[0]
    out = np.zeros((NC, 16, len(calls) * GC * 8), a.dtype)
    for ci, (_, t0, nt) in enumerate(calls):
        blk = a[:, t0 * 128:(t0 + nt) * 128]
        w = blk.reshape(NC, nt * 8, 16).transpose(0, 2, 1)
        out[:, :, ci * GC * 8:ci * GC * 8 + nt * 8] = w
    return out


def _pm_stream(a, calls, GC):
    """[NC, E, K] -> [NC, 128, ncalls*GC*K]: edge (t*128+p) of call ci, lane k
    at [p, (ci*GC+t)*K + k]."""
    NC, _, K = a.shape
    out = np.zeros((NC, 128, len(calls) * GC * K), a.dtype)
    for ci, (_, t0, nt) in enumerate(calls):
        blk = a[:, t0 * 128:(t0 + nt) * 128, :]
        w = blk.reshape(NC, nt, 128, K).transpose(0, 2, 1, 3).reshape(
            NC, 128, nt * K)
        out[:, :, ci * GC * K:ci * GC * K + nt * K] = w
    return out


def _build(s, H=64, OUT=16, B=8, collectives=True):
    import concourse.bass as bass  # noqa: F401
    import concourse.bacc as bacc
    import concourse.mybir as mybir
    from concourse.tile import TileContext

    f32 = mybir.dt.float32
    bf16 = mybir.dt.bfloat16
    i16 = mybir.dt.int16
    AF = mybir.ActivationFunctionType
    NC, GC = s.NC, s.GC
    WN, NW = s.WN, s.NW
    BO = B * H                      # 512-wide basis block
    nA = len(s.calls)

    nc = bacc.Bacc(None, target_bir_lowering=False)

    feats = nc.declare_dram_parameter("feats", [s.N, H], f32, isOutput=False)
    BBp = [nc.declare_dram_parameter(f"BB{l}", [H, BO], bf16, isOutput=False)
           for l in range(3)]
    cnS = [nc.declare_dram_parameter(f"cnS{l}", [128, nA * GC * B], f32,
                                     isOutput=False) for l in range(3)]
    gidxS = nc.declare_dram_parameter("gidxS", [16, nA * GC * 8], i16,
                                      isOutput=False)
    dlS = nc.declare_dram_parameter("dlS", [128, nA * GC], f32, isOutput=False)
    outs = nc.declare_dram_parameter("outs", [s.NPC, OUT], f32, isOutput=True)

    hslice = [nc.dram_tensor(f"hslice{l}", [s.NPC, H], f32) for l in range(2)]
    hfull = [nc.dram_tensor(f"hfull{l}", [s.N, H], f32, addr_space="Shared")
             for l in range(2)]

    for l in range(3):
        OW = OUT if l == 2 else H
        xsrc = feats if l == 0 else hfull[l - 1]
        with TileContext(nc) as tc:
            with (
                tc.tile_pool(name="const", bufs=1) as pconst,
                tc.tile_pool(name="wstage", bufs=1) as pw,
                tc.tile_pool(name="idx", bufs=2) as pidx,
                tc.tile_pool(name="meta", bufs=2) as pmeta,
                tc.tile_pool(name="gx", bufs=3) as pgx,
                tc.tile_pool(name="xt", bufs=3) as pxt,
                tc.tile_pool(name="xtp", bufs=2, space="PSUM") as pxtps,
                tc.tile_pool(name="pp", bufs=2, space="PSUM") as pmps,
                tc.tile_pool(name="tt", bufs=3) as pT,
                tc.tile_pool(name="oh", bufs=4) as poh,
                tc.tile_pool(name="hps", bufs=2, space="PSUM") as phps,
                tc.tile_pool(name="red", bufs=2) as prd,
                tc.tile_pool(name="ost", bufs=2) as post,
            ):
                iota = pconst.tile([128, WN], f32, tag="iota")
                nc.gpsimd.iota(iota[:], [[1, WN]], base=0, channel_multiplier=0,
                               allow_small_or_imprecise_dtypes=True)
                iop = pconst.tile([128, 1], f32, tag="iop")
                nc.gpsimd.iota(iop[:], [[0, 1]], base=0, channel_multiplier=1,
                               allow_small_or_imprecise_dtypes=True)
                iof = pconst.tile([128, 128], f32, tag="iof")
                nc.gpsimd.iota(iof[:], [[1, 128]], base=0, channel_multiplier=0,
                               allow_small_or_imprecise_dtypes=True)
                ident = pconst.tile([128, 128], f32, tag="ident")
                nc.vector.tensor_scalar(ident[:], iof[:], iop[:], None,
                                        op0=mybir.AluOpType.is_equal)

                BBs = pw.tile([128, BO], bf16, tag="bb")
                nc.sync.dma_start(out=BBs[:H, :], in_=BBp[l][:])
                nc.sync.dma_start(out=BBs[H:2 * H, :], in_=BBp[l][:])

                KB = 16
                WB = next(d for d in (5, 4, 2, 1) if NW % d == 0)
                itb = cnb = dlb = None
                hp = ob = None
                for ci, (ch, t0, nt) in enumerate(s.calls):
                    if ci % KB == 0:
                        n_in_b = min(KB, nA - ci)
                        itb = pidx.tile([128, KB * GC * 8], i16, tag="it")
                        for rp in range(8):
                            nc.sync.dma_start(
                                out=itb[rp * 16:(rp + 1) * 16, :n_in_b * GC * 8],
                                in_=gidxS[:, ci * GC * 8:(ci + n_in_b) * GC * 8])
                        cnb = pmeta.tile([128, KB * GC * B], f32, tag="cn")
                        nc.sync.dma_start(
                            out=cnb[:, :n_in_b * GC * B],
                            in_=cnS[l][:, ci * GC * B:(ci + n_in_b) * GC * B])
                        dlb = pmeta.tile([128, KB * GC], f32, tag="dl")
                        nc.sync.dma_start(
                            out=dlb[:, :n_in_b * GC],
                            in_=dlS[:, ci * GC:(ci + n_in_b) * GC])
                    it = itb[:, (ci % KB) * GC * 8:(ci % KB) * GC * 8 + nt * 8]
                    gt = pgx.tile([128, GC, H], f32, tag="gt")
                    nc.gpsimd.dma_gather(
                        gt[:, :nt, :], xsrc[ch * s.XCH:ch * s.XCH + s.XCH],
                        it, num_idxs=nt * 128, num_idxs_reg=nt * 128,
                        elem_size=H)
                    for k in range(0, nt, 2):
                        n2 = min(2, nt - k)
                        xtp = pxtps.tile([128, 128], f32, tag="xtp")
                        if n2 == 2:
                            nc.tensor.transpose(
                                xtp[:],
                                gt[:, k:k + 2, :].rearrange("p a b -> p (a b)"),
                                ident[:])
                        else:
                            nc.tensor.transpose(xtp[:H, :], gt[:, k, :],
                                                ident[:])
                        xt = pxt.tile([128, 128], bf16, tag="xt")
                        nc.scalar.activation(xt[:n2 * H, :], xtp[:n2 * H, :],
                                             AF.Copy)
                        for j in range(n2):
                            t = t0 + k + j
                            kk = (ci % KB) * GC + k + j
                            w = int(s.window_of[t])
                            first = t == int(s.wst[w])
                            last = t == int(s.wst[w + 1]) - 1
                            P = pmps.tile([128, BO], f32, tag="pp")
                            nc.tensor.matmul(
                                P[:, :], xt[j * H:(j + 1) * H, :],
                                BBs[j * H:(j + 1) * H, :],
                                start=True, stop=True)
                            T = pT.tile([128, BO], bf16, tag="tt")
                            cnv = cnb[:, kk * B:(kk + 1) * B].rearrange(
                                "p (b u) -> p b u", u=1).to_broadcast(
                                [128, B, H])
                            nc.vector.tensor_tensor(
                                out=T.rearrange("p (b o) -> p b o", o=H),
                                in0=P.rearrange("p (b o) -> p b o", o=H),
                                in1=cnv, op=mybir.AluOpType.mult)
                            oh = poh.tile([128, WN], bf16, tag="oh")
                            nc.gpsimd.tensor_scalar(
                                oh[:], iota[:], dlb[:, kk:kk + 1], None,
                                op0=mybir.AluOpType.is_equal)
                            if first:
                                hp = phps.tile([128, BO], f32, tag="hp",
                                               name=f"hp{l}_{w}")
                            nc.tensor.matmul(hp[:WN, :], oh[:, :WN], T[:, :],
                                             start=first, stop=last)
                            if last:
                                r = prd.tile([128, H], f32, tag="red")
                                nc.vector.tensor_reduce(
                                    r[:WN, :],
                                    hp[:WN, :].rearrange(
                                        "p (b o) -> p o b", o=H),
                                    axis=mybir.AxisListType.X,
                                    op=mybir.AluOpType.add)
                                if w % WB == 0:
                                    ob = post.tile([128, WB, H], f32,
                                                   tag="ob")
                                nc.scalar.activation(
                                    ob[:WN, w % WB, :OW], r[:WN, :OW],
                                    AF.Relu if l < 2 else AF.Copy)
                                if w % WB == WB - 1:
                                    w0 = w - (WB - 1)
                                    dstt = outs if l == 2 else hslice[l]
                                    nc.sync.dma_start(
                                        out=dstt.ap()[w0 * WN:(w0 + WB) * WN, :]
                                            .rearrange("(a p) f -> p a f",
                                                       p=WN),
                                        in_=ob[:WN, :, :OW])

        if l < 2:
            with nc.semaphore(f"ccsem{l}") as cc_sem:
                if collectives:
                    nc.gpsimd.collective_compute(
                        "AllGather", mybir.AluOpType.bypass,
                        replica_groups=[list(range(NC))],
                        ins=[hslice[l][:]], outs=[hfull[l][:]],
                    ).then_inc(cc_sem)
                    nc.gpsimd.wait_ge(cc_sem, 1)
                else:
                    nc.gpsimd.dma_start(out=hfull[l][0:s.NPC, :],
                                        in_=hslice[l][:]).then_inc(cc_sem, 16)
                    nc.gpsimd.wait_ge(cc_sem, 16)
            nc.all_engine_barrier()

    nc.finalize()
    return nc


def _to_bf16(a):
    import ml_dtypes
    return np.ascontiguousarray(a.astype(ml_dtypes.bfloat16))


def _make_in_maps(inputs, s, H):
    feats = np.ascontiguousarray(np.asarray(inputs["feats"], np.float32))
    etype = np.asarray(inputs["etype"]).astype(np.int64).ravel()
    norm = np.asarray(inputs["norm"], np.float32).ravel()
    B = np.asarray(inputs["coeff0"]).shape[1]
    NC, EA, GC = s.NC, s.EA, s.GC

    base = {}
    for l in range(3):
        bs = np.asarray(inputs[f"bases{l}"], np.float32)
        O = bs.shape[2]
        if O < H:
            bs = np.pad(bs, ((0, 0), (0, 0), (0, H - O)))
        base[f"BB{l}"] = _to_bf16(bs.transpose(1, 0, 2).reshape(H, B * H))
        cnfull = (np.asarray(inputs[f"coeff{l}"], np.float32)[etype]
                  * norm[:, None])
        cnarr = np.zeros((NC, EA, B), np.float32)
        for c in range(NC):
            eo, pos = s.slots[c]
            cnarr[c, pos] = cnfull[eo]
        base[f"cnS{l}"] = _pm_stream(cnarr, s.calls, GC)
    for l in range(3):
        bias = np.asarray(inputs[f"bias{l}"], np.float32)
        assert not bias.any(), "nonzero bias unsupported by this kernel"
    return [
        {"feats": feats,
         "BB0": base["BB0"], "BB1": base["BB1"], "BB2": base["BB2"],
         "cnS0": base["cnS0"][c], "cnS1": base["cnS1"][c],
         "cnS2": base["cnS2"][c],
         "gidxS": s.gidxS[c], "dlS": s.dlS[c]}
        for c in range(NC)
    ]


_CACHE = {}


def _get_compiled(inputs):
    feats = np.asarray(inputs["feats"], np.float32)
    N, H = feats.shape
    OUT = np.asarray(inputs["bases2"]).shape[2]
    R, B = np.asarray(inputs["coeff0"]).shape
    key = (N, H, OUT, R, B)
    if key not in _CACHE:
        s = _prepare(inputs["src"], inputs["dst"], N=N, NC=8)
        nc = _build(s, H=H, OUT=OUT, B=B, collectives=True)
        _CACHE[key] = (s, nc)
    return _CACHE[key]


def kernel(**inputs):
    """Full-input, full-output 3-layer RGCN on 8 NeuronCores."""
    from concourse.bass_utils import run_bass_kernel_spmd

    H = np.asarray(inputs["feats"]).shape[1]
    s, nc = _get_compiled(inputs)
    in_maps = _make_in_maps(inputs, s, H)
    res = run_bass_kernel_spmd(nc, in_maps, list(range(s.NC)))
    return np.concatenate([res.results[c]["outs"] for c in range(s.NC)], axis=0)


if __name__ == "__main__":
    rng = np.random.default_rng(0)
    N, E, H, OUT, R, B = 100000, 1000000, 64, 16, 90, 8
    inputs = dict(
        feats=rng.standard_normal((N, H)).astype(np.float32),
        src=rng.integers(0, N, E), dst=rng.integers(0, N, E),
        etype=rng.integers(0, R, E),
        norm=rng.random((E, 1)).astype(np.float32),
    )
    for l, o in enumerate([H, H, OUT]):
        inputs[f"coeff{l}"] = (rng.standard_normal((R, B)) / np.sqrt(B)).astype(np.float32)
        inputs[f"bases{l}"] = (rng.standard_normal((B, H, o)) / np.sqrt(H)).astype(np.float32)
        inputs[f"bias{l}"] = np.zeros(o, np.float32)
    out = kernel(**inputs)
    print("kernel out", out.shape, out.dtype, float(np.abs(out).max()))
